# revision 10
# baseline (speedup 1.0000x reference)
"""LSTM encoder kernel for Trainium2 (Bass/Tile), 8-core data-parallel.

Problem: nn_Encoder (B=32, T=2048, E=512, H=512), keras-order LSTM cell:
    z = x_t @ W + h @ U + b ; i,f,g,o = split(z); c' = sig(f)*c + sig(i)*tanh(g);
    h' = sig(o)*tanh(c')
Returns (h_T, hs[T,B,H], h_T).

Sharding: batch dim (32) split across 8 cores (4 rows each); W/U/b replicated.
Per core the time scan is local and fully serial.

Device algorithm per core (all fp32):
  - Gate columns of W/U/b are host-permuted into "pair" layout so the
    per-step activations are 2 big contiguous ACT ops per half:
      pair p in {0,1}: [ i_p(256) | f_p(256) | o_p(256) | g_p(256) ]
    where x_p means columns [256p, 256p+256) of that gate.
  - src is host-transposed to srcT [E, T*4] (token = t*4 + b_local).
  - Window loop (128 tokens = 32 steps per window):
      xw_win[tok,2048] = srcT_win.T @ W + b  (dense matmuls, PSUM->SBUF)
      32 serial steps:
        z[0:4,2048](PSUM)  = E_j.T @ xw_win   (identity-slice inject, start=True)
                           += hT_k.T @ U_k    (4x4 accumulating matmuls)
        gates: sigmoid/tanh (ScalarE, PSUM src), c/h update (VectorE),
        h -> hs buffer, PE-transpose h into hT chunks for the next step.
      hs buffer DMA'd out once per window.
"""

import numpy as np

B, T, E, H = 32, 2048, 512, 512
NCORES = 8
BL = B // NCORES            # 4 batch rows per core
G4 = 4 * H                  # 2048 gate columns
TOK = T * BL                # 8192 tokens per core
WIN = 128                   # tokens per window
SPW = WIN // BL             # 32 steps per window
NWIN = TOK // WIN
PAIR = 2                    # gate pairs
HP = H // PAIR              # 256 h-cols per pair
QK = H // 128               # 4 contraction chunks


def _gate_perm():
    """Permutation of the 4H gate columns into pair layout."""
    idx = np.arange(H)
    perm = []
    for p in range(PAIR):
        cols = idx[HP * p : HP * (p + 1)]
        perm.extend(0 * H + cols)   # i_p
        perm.extend(1 * H + cols)   # f_p
        perm.extend(3 * H + cols)   # o_p
        perm.extend(2 * H + cols)   # g_p
    return np.asarray(perm)


def _build_bass(ntok, f32r=True):
    """Build the per-core Bass program for ntok tokens (ntok % WIN == 0)."""
    import concourse.bass as bass
    import concourse.mybir as mybir
    import concourse.tile as tile
    from concourse import bacc
    from concourse.bass import ds
    from concourse.masks import make_identity

    f32 = mybir.dt.float32

    if f32r:
        def R(ap):
            return ap.bitcast(mybir.dt.float32r)
    else:
        def R(ap):
            return ap
    AF = mybir.ActivationFunctionType
    nwin = ntok // WIN

    nc = bacc.Bacc()
    srcT = nc.dram_tensor("srcT", [E, ntok], f32, kind="ExternalInput")
    Wp = nc.dram_tensor("Wp", [E, G4], f32, kind="ExternalInput")
    Up = nc.dram_tensor("Up", [H, G4], f32, kind="ExternalInput")
    bp = nc.dram_tensor("bp", [1, G4], f32, kind="ExternalInput")
    h0T = nc.dram_tensor("h0T", [H, BL], f32, kind="ExternalInput")
    c0 = nc.dram_tensor("c0", [BL, H], f32, kind="ExternalInput")
    hs = nc.dram_tensor("hs", [ntok, H], f32, kind="ExternalOutput")

    with tile.TileContext(nc) as tc:
        with (
            tc.tile_pool(name="constp", bufs=1) as constp,
            tc.tile_pool(name="statep", bufs=1) as statep,
            tc.tile_pool(name="iop", bufs=2) as iop,
            tc.tile_pool(name="gatep", bufs=2) as gatep,
            tc.tile_pool(name="zp", bufs=1, space="PSUM") as zp,
            tc.tile_pool(name="xwp", bufs=2, space="PSUM") as xwpp,
            tc.tile_pool(name="tp", bufs=2, space="PSUM") as tpp,
        ):
            # ---- resident constants ----
            Ut = []
            for k in range(QK):
                ut = constp.tile([128, G4], f32, tag=f"U{k}", name=f"Ut{k}")
                nc.sync.dma_start(ut, Up[128 * k : 128 * (k + 1), :])
                Ut.append(ut)
            Wt = []
            for k in range(QK):
                wt = constp.tile([128, G4], f32, tag=f"W{k}", name=f"Wt{k}")
                nc.sync.dma_start(wt, Wp[128 * k : 128 * (k + 1), :])
                Wt.append(wt)
            bt = constp.tile([1, G4], f32, tag="bt", name="bt")
            nc.sync.dma_start(bt, bp[:, :])
            ones = constp.tile([1, WIN], f32, tag="ones", name="ones")
            nc.vector.memset(ones, 1.0)
            ident = constp.tile([128, 128], f32, tag="ident", name="ident")
            make_identity(nc, ident)

            # ---- state ----
            hT = []
            for k in range(QK):
                t_ = statep.tile([128, BL], f32, tag=f"hT{k}", name=f"hT{k}")
                nc.sync.dma_start(t_, h0T[128 * k : 128 * (k + 1), :])
                hT.append(t_)
            cst = []
            for p in range(PAIR):
                t_ = statep.tile([BL, HP], f32, tag=f"c{p}", name=f"cst{p}")
                nc.sync.dma_start(t_, c0[:, HP * p : HP * (p + 1)])
                cst.append(t_)

            # persistent PSUM z accumulator [BL, 2048] (4 banks)
            zt = zp.tile([BL, G4], f32, tag="z", name="zt")

            hints = (
                mybir.EngineType.PE,
                mybir.EngineType.DVE,
                mybir.EngineType.Activation,
            )
            with tc.For_i(
                0, ntok, WIN, hint_engines=hints, staggered_reset=True
            ) as tok0:
                # ---- window phase: xw_win = srcT_win.T @ W + b ----
                sT = []
                for k in range(QK):
                    t_ = iop.tile([128, WIN], f32, tag=f"sT{k}", name=f"sT{k}")
                    nc.sync.dma_start(
                        t_, srcT[128 * k : 128 * (k + 1), ds(tok0, WIN)]
                    )
                    sT.append(t_)
                xw_win = iop.tile([WIN, G4], f32, tag="xw", name="xw_win")
                for t4 in range(4):
                    nsl = slice(512 * t4, 512 * (t4 + 1))
                    xwp = xwpp.tile([WIN, 512], f32, tag="xwp", name="xwp")
                    for k in range(QK):
                        nc.tensor.matmul(
                            xwp, R(sT[k]), R(Wt[k][:, nsl]),
                            start=(k == 0), stop=False,
                        )
                    nc.tensor.matmul(
                        xwp, R(ones), R(bt[:, nsl]), start=False, stop=True
                    )
                    nc.vector.tensor_copy(xw_win[:, nsl], xwp)

                # h staging: step-major along free dim, partitions = batch
                hsb = iop.tile([BL, SPW * H], f32, tag="hsb", name="hsb", bufs=1)

                # ---- 32 serial recurrence steps ----
                for j in range(SPW):
                    ej = ident[:, BL * j : BL * (j + 1)]
                    # inject xw for this step: z[b, :] = xw_win[4j+b, :]
                    for t4 in range(4):
                        nsl = slice(512 * t4, 512 * (t4 + 1))
                        nc.tensor.matmul(
                            zt[:, nsl], R(ej), R(xw_win[:, nsl]),
                            start=True, stop=False,
                        )
                    # recurrent matmuls: z += h @ U
                    for t4 in range(4):
                        nsl = slice(512 * t4, 512 * (t4 + 1))
                        for k in range(QK):
                            nc.tensor.matmul(
                                zt[:, nsl], R(hT[k]), R(Ut[k][:, nsl]),
                                start=False, stop=(k == QK - 1),
                            )
                    # gates + state update, per pair
                    for p in range(PAIR):
                        base = 1024 * p
                        sig = gatep.tile([BL, 768], f32, tag=f"sig{p}",
                                         name=f"sig{p}")
                        tg = gatep.tile([BL, HP], f32, tag=f"tg{p}",
                                        name=f"tg{p}")
                        nc.scalar.activation(
                            sig, zt[:, base : base + 768], AF.Sigmoid
                        )
                        nc.scalar.activation(
                            tg, zt[:, base + 768 : base + 1024], AF.Tanh
                        )
                        fc = gatep.tile([BL, HP], f32, tag=f"fc{p}", name=f"fc{p}")
                        ig = gatep.tile([BL, HP], f32, tag=f"ig{p}", name=f"ig{p}")
                        nc.vector.tensor_mul(fc, sig[:, 256:512], cst[p])
                        nc.vector.tensor_mul(ig, sig[:, 0:256], tg)
                        nc.vector.tensor_add(cst[p], fc, ig)
                        tcp = gatep.tile([BL, HP], f32, tag=f"tc{p}", name=f"tc{p}")
                        nc.scalar.activation(tcp, cst[p], AF.Tanh)
                        hoff = H * j + HP * p
                        nc.vector.tensor_mul(
                            hsb[:, hoff : hoff + HP], sig[:, 512:768], tcp
                        )
                        # transpose h chunks into hT for the next step
                        for q2 in range(2):
                            q = 2 * p + q2
                            pt = tpp.tile([128, BL], f32, tag="pt", name="pt")
                            nc.tensor.transpose(
                                pt,
                                hsb[:, hoff + 128 * q2 : hoff + 128 * (q2 + 1)],
                                ident[:BL, :BL],
                            )
                            nc.vector.tensor_copy(hT[q], pt)

                nc.sync.dma_start(
                    hs[ds(tok0, WIN), :].rearrange("(j b) e -> b j e", b=BL),
                    hsb.rearrange("b (j e) -> b j e", e=H),
                )

    nc.finalize()
    return nc


_CACHE = {}


def _get_nc(ntok, f32r=True):
    key = (ntok, f32r)
    if key not in _CACHE:
        _CACHE[key] = _build_bass(ntok, f32r=f32r)
    return _CACHE[key]


def _prep_inputs(src_seq, W, U, b, h0, c0, ntok=TOK):
    """Host-side shard + permute. Returns in_maps for run_bass_kernel_spmd."""
    perm = _gate_perm()
    Wp = np.ascontiguousarray(np.asarray(W, np.float32)[:, perm])
    Up = np.ascontiguousarray(np.asarray(U, np.float32)[:, perm])
    bp = np.ascontiguousarray(np.asarray(b, np.float32)[perm][None, :])
    src = np.asarray(src_seq, np.float32)
    h0 = np.asarray(h0, np.float32)
    c0 = np.asarray(c0, np.float32)
    t_used = ntok // BL
    in_maps = []
    for j in range(NCORES):
        rows = slice(BL * j, BL * (j + 1))
        # srcT[e, t*BL + b] = src[b, t, e]
        srcT = np.ascontiguousarray(
            src[rows, :t_used, :].transpose(2, 1, 0).reshape(E, ntok)
        )
        in_maps.append(
            {
                "srcT": srcT,
                "Wp": Wp,
                "Up": Up,
                "bp": bp,
                "h0T": np.ascontiguousarray(h0[rows].T),
                "c0": np.ascontiguousarray(c0[rows]),
            }
        )
    return in_maps


def _assemble(results, ntok=TOK):
    t_used = ntok // BL
    hs = np.concatenate(
        [np.asarray(r["hs"]).reshape(t_used, BL, H) for r in results], axis=1
    )
    h_T = hs[-1]
    return h_T, hs, h_T


def kernel(src_seq, W, U, b, h0, c0):
    from concourse import bass_utils

    nc = _get_nc(TOK)
    in_maps = _prep_inputs(src_seq, W, U, b, h0, c0)
    res = bass_utils.run_bass_kernel_spmd(
        nc, in_maps, core_ids=list(range(NCORES))
    )
    return _assemble(res.results)
